# revision 13
# baseline (speedup 1.0000x reference)
"""Trainium2 Bass kernel for MeshConv-style GNN message passing.

Pipeline (per edge e with src s, dst d):
    feat = [x[d], x[s], edge_attr[e]]           # [2*128+4]
    h    = feat @ W1 + b1                       # [128]
    h    = group_norm(h, gamma, beta)           # 8 groups of 16
    h    = silu(h)
    msg  = h @ W2 + b2
    out[n] = sum_{e: dst=n} msg[e] / max(count[n], 1)

Sharding: edges are sorted by dst and partitioned so that each of the 8
cores owns a contiguous 12,500-node slice of the output.  Within a core,
nodes are grouped into 128-node windows; each window's edges are padded
to a multiple of 128 (one "tile" = 128 edges).  A tile's contribution to
its window is computed as S_T.T @ [h_sil | 1] where S_T[e, n] =
(dst_local[e] == n) is built on-chip with an iota compare; the per-window
PSUM accumulator therefore collects both the message sums and the
counts.  W2 is applied once per window AFTER the edge reduction
(associativity: (S@h)@W2 == S@(h@W2)), which removes a per-tile matmul
and the transpose of h.  No cross-core collective is needed.
"""

import sys

if "/opt/trn_rl_repo" not in sys.path:
    sys.path.insert(0, "/opt/trn_rl_repo")

import numpy as np

N_NODES = 100000
IN_DIM = 128
OUT_DIM = 128
EDGE_DIM = 4
N_GROUPS = 8
GSIZE = IN_DIM // N_GROUPS  # 16
EPS = 1e-5

N_CORES = 8
NPC = N_NODES // N_CORES          # nodes per core (12500)
WIN = 128                         # nodes per window
TE = 128                          # edges per tile

LAST_EXEC_NS = None
LAST_RESULTS = None
# CoreSim lacks Silu; set True to emit Sigmoid+mult instead (sim testing only)
SIM_SAFE_SILU = False
# dump per-stage intermediates of tile 0 / window 0 to DRAM (debugging)
DEBUG_DUMP = False


def _shard(x, edge_index, edge_attr):
    """Sort edges by dst, partition by core / window, pad to tiles.

    Returns (T_ws, per_core) where T_ws[w] = tiles in window w (same for
    every core) and per_core[c] = dict of device input arrays.
    """
    src = np.ascontiguousarray(edge_index[0]).astype(np.int64)
    dst = np.ascontiguousarray(edge_index[1]).astype(np.int64)
    E = src.shape[0]
    ea = np.ascontiguousarray(edge_attr).astype(np.float32)

    order = np.argsort(dst, kind="stable")
    src = src[order]
    dst = dst[order]
    ea = ea[order]

    core = dst // NPC
    core = np.minimum(core, N_CORES - 1)
    local = dst - core * NPC
    win = local >> 7
    nwin = (NPC + WIN - 1) // WIN  # 98

    # edges per (core, window)
    cw = core * nwin + win
    counts = np.bincount(cw, minlength=N_CORES * nwin).reshape(N_CORES, nwin)
    T_ws = np.maximum(1, (counts.max(axis=0) + TE - 1) // TE).astype(np.int64)
    total_tiles = int(T_ws.sum())
    cap = total_tiles * TE

    # slot offset of each window in the padded edge array
    woff = np.zeros(nwin, dtype=np.int64)
    woff[1:] = np.cumsum(T_ws)[:-1] * TE

    # position of each edge within its (core, window) run
    cw_starts = np.zeros(N_CORES * nwin, dtype=np.int64)
    cw_starts[1:] = np.cumsum(counts.reshape(-1))[:-1]
    pos_in_cw = np.arange(E, dtype=np.int64) - cw_starts[cw]

    slot = woff[win] + pos_in_cw  # per-core slot index

    per_core = []
    for c in range(N_CORES):
        m = core == c
        sl = slot[m]
        idx = np.zeros((cap, 2), dtype=np.int32)
        dsh = np.full((cap, 1), -1.0, dtype=np.float32)
        eat = np.zeros((total_tiles, 5, TE), dtype=np.float32)
        eat[:, 4, :] = 1.0  # ones row (b1 term); pads harmless (S_T kills them)

        idx[sl, 0] = src[m].astype(np.int32)
        idx[sl, 1] = dst[m].astype(np.int32)
        dsh[sl, 0] = (local[m] - (win[m] << 7)).astype(np.float32)
        t_of_slot = sl // TE
        e_of_slot = sl % TE
        eat[t_of_slot, 0:4, e_of_slot] = ea[m]
        per_core.append({"idx": idx, "dsh": dsh,
                         "eat": eat.reshape(total_tiles * 5, TE)})
    return T_ws, per_core


def _build_program(T_ws, trivial_affine):
    import concourse.bacc as bacc
    import concourse.bass as bass
    from concourse import mybir
    from concourse.tile import TileContext

    f32 = mybir.dt.float32
    i32 = mybir.dt.int32
    AF = mybir.ActivationFunctionType
    OP = mybir.AluOpType
    AX = mybir.AxisListType

    nwin = len(T_ws)
    total_tiles = int(sum(T_ws))

    nc = bacc.Bacc()
    x_d = nc.dram_tensor("x", [N_NODES, IN_DIM], f32, kind="ExternalInput")
    idx_d = nc.dram_tensor("idx", [total_tiles * TE, 2], i32, kind="ExternalInput")
    dsh_d = nc.dram_tensor("dsh", [total_tiles * TE, 1], f32, kind="ExternalInput")
    eat_d = nc.dram_tensor("eat", [total_tiles * 5, TE], f32, kind="ExternalInput")
    w1a_d = nc.dram_tensor("w1a", [128, 128], f32, kind="ExternalInput")
    w1b_d = nc.dram_tensor("w1b", [128, 128], f32, kind="ExternalInput")
    w1e_d = nc.dram_tensor("w1e", [5, 128], f32, kind="ExternalInput")
    w2_d = nc.dram_tensor("w2", [128, 128], f32, kind="ExternalInput")
    b2_d = nc.dram_tensor("b2t", [128, 128], f32, kind="ExternalInput")
    iota_d = nc.dram_tensor("iota", [128, 128], f32, kind="ExternalInput")
    id_d = nc.dram_tensor("ident", [128, 128], f32, kind="ExternalInput")
    if not trivial_affine:
        gma_d = nc.dram_tensor("gmat", [128, 128], f32, kind="ExternalInput")
        bta_d = nc.dram_tensor("btat", [128, 128], f32, kind="ExternalInput")
    out_d = nc.dram_tensor("out", [nwin * WIN, OUT_DIM], f32, kind="ExternalOutput")
    if DEBUG_DUMP:
        dbg = {
            name: nc.dram_tensor(name, shape, f32, kind="ExternalOutput")
            for name, shape in {
                "dbg_xg": [128, 256], "dbg_st": [128, 128],
                "dbg_h": [128, 128], "dbg_z": [128, 128],
                "dbg_hs": [128, 129], "dbg_us": [128, 129],
            }.items()
        }

    with TileContext(nc) as tc:
        with (
            tc.tile_pool(name="const", bufs=1) as cp,
            tc.tile_pool(name="sb", bufs=4) as sb,
            tc.tile_pool(name="wp", bufs=2) as wp,
            tc.tile_pool(name="ps", bufs=2, space="PSUM") as ps,
            tc.tile_pool(name="psw", bufs=2, space="PSUM") as psw,
        ):
            def cload(dram, shape, tag):
                t = cp.tile(shape, f32, tag=tag)
                nc.sync.dma_start(out=t[:], in_=dram[:])
                return t

            W1A = cload(w1a_d, [128, 128], "c_w1a")
            W1B = cload(w1b_d, [128, 128], "c_w1b")
            W1E = cload(w1e_d, [5, 128], "c_w1e")
            W2 = cload(w2_d, [128, 128], "c_w2")
            B2T = cload(b2_d, [128, 128], "c_b2t")
            IOTA = cload(iota_d, [128, 128], "c_iota")
            IDENT = cload(id_d, [128, 128], "c_id")
            if not trivial_affine:
                GMAT = cload(gma_d, [128, 128], "c_gma")
                BTAT = cload(bta_d, [128, 128], "c_bta")

            EPS_T = cp.tile([128, 1], f32, tag="c_eps")
            nc.vector.memset(EPS_T[:], EPS)
            SC16_T = cp.tile([128, 1], f32, tag="c_sc16")
            nc.vector.memset(SC16_T[:], 1.0 / GSIZE)

            gt = 0
            for w in range(nwin):
                u_w = psw.tile([128, 129], f32, tag="uw")
                Tw = int(T_ws[w])
                for t in range(Tw):
                    base = gt * TE
                    idx_t = sb.tile([128, 2], i32, tag="idx")
                    nc.sync.dma_start(out=idx_t[:], in_=idx_d[base:base + TE, :])
                    dsh_t = sb.tile([128, 1], f32, tag="dsh")
                    nc.sync.dma_start(out=dsh_t[:], in_=dsh_d[base:base + TE, :])
                    eat_t = sb.tile([5, 128], f32, tag="eat")
                    nc.sync.dma_start(out=eat_t[:], in_=eat_d[gt * 5:(gt + 1) * 5, :])

                    xg = sb.tile([128, 256], f32, tag="xg")
                    nc.gpsimd.indirect_dma_start(
                        out=xg[:, 0:128], out_offset=None, in_=x_d[:],
                        in_offset=bass.IndirectOffsetOnAxis(ap=idx_t[:, 0:1], axis=0),
                    )
                    nc.gpsimd.indirect_dma_start(
                        out=xg[:, 128:256], out_offset=None, in_=x_d[:],
                        in_offset=bass.IndirectOffsetOnAxis(ap=idx_t[:, 1:2], axis=0),
                    )

                    xsT_p = ps.tile([128, 128], f32, tag="tp")
                    nc.tensor.transpose(xsT_p[:], xg[:, 0:128], IDENT[:])
                    xsT = sb.tile([128, 128], f32, tag="xsT")
                    nc.scalar.copy(out=xsT[:], in_=xsT_p[:])
                    xdT_p = ps.tile([128, 128], f32, tag="tp")
                    nc.tensor.transpose(xdT_p[:], xg[:, 128:256], IDENT[:])
                    xdT = sb.tile([128, 128], f32, tag="xdT")
                    nc.scalar.copy(out=xdT[:], in_=xdT_p[:])

                    # S_T[e, n] = (dst_local[e] == n)
                    st = sb.tile([128, 128], f32, tag="st")
                    nc.vector.tensor_tensor(
                        out=st[:], in0=dsh_t[:].to_broadcast([128, 128]),
                        in1=IOTA[:], op=OP.is_equal,
                    )

                    h_p = ps.tile([128, 128], f32, tag="h")
                    nc.tensor.matmul(h_p[:], lhsT=xdT[:], rhs=W1A[:], start=True, stop=False)
                    nc.tensor.matmul(h_p[:], lhsT=xsT[:], rhs=W1B[:], start=False, stop=False)
                    nc.tensor.matmul(h_p[:], lhsT=eat_t[:], rhs=W1E[:], start=False, stop=True)

                    hg = h_p[:].rearrange("p (g c) -> p g c", g=N_GROUPS)
                    sq = sb.tile([128, 128], f32, tag="sq")
                    nc.scalar.activation(out=sq[:], in_=h_p[:], func=AF.Square)
                    s1 = sb.tile([128, 8], f32, tag="s1")
                    nc.vector.tensor_reduce(out=s1[:], in_=hg, axis=AX.X, op=OP.add)
                    s2 = sb.tile([128, 8], f32, tag="s2")
                    nc.vector.tensor_reduce(
                        out=s2[:], in_=sq[:].rearrange("p (g c) -> p g c", g=N_GROUPS),
                        axis=AX.X, op=OP.add,
                    )
                    # var*16 = s2 - s1^2/16 ; sd = sqrt(var + eps) ; inv = 1/sd
                    d2 = sb.tile([128, 8], f32, tag="d2")
                    nc.vector.tensor_tensor(out=d2[:], in0=s1[:], in1=s1[:], op=OP.mult)
                    v16 = sb.tile([128, 8], f32, tag="v16")
                    nc.vector.scalar_tensor_tensor(
                        out=v16[:], in0=d2[:], scalar=-1.0 / GSIZE, in1=s2[:],
                        op0=OP.mult, op1=OP.add,
                    )
                    sd = sb.tile([128, 8], f32, tag="sd")
                    nc.scalar.activation(out=sd[:], in_=v16[:], func=AF.Sqrt,
                                         scale=SC16_T[:], bias=EPS_T[:])
                    inv = sb.tile([128, 8], f32, tag="inv")
                    nc.vector.reciprocal(out=inv[:], in_=sd[:])

                    # z = (h - mean) * inv
                    z1 = sb.tile([128, 128], f32, tag="z1")
                    nc.vector.scalar_tensor_tensor(
                        out=z1[:].rearrange("p (g c) -> p g c", g=N_GROUPS),
                        in0=s1[:, :, None].to_broadcast([128, 8, GSIZE]),
                        scalar=-1.0 / GSIZE, in1=hg,
                        op0=OP.mult, op1=OP.add,
                    )
                    z = sb.tile([128, 128], f32, tag="z")
                    nc.vector.tensor_tensor(
                        out=z[:].rearrange("p (g c) -> p g c", g=N_GROUPS),
                        in0=z1[:].rearrange("p (g c) -> p g c", g=N_GROUPS),
                        in1=inv[:, :, None].to_broadcast([128, 8, GSIZE]),
                        op=OP.mult,
                    )
                    if not trivial_affine:
                        nc.vector.tensor_tensor(out=z[:], in0=z[:], in1=GMAT[:], op=OP.mult)
                        nc.vector.tensor_tensor(out=z[:], in0=z[:], in1=BTAT[:], op=OP.add)

                    hs = sb.tile([128, 129], f32, tag="hs")
                    nc.gpsimd.memset(hs[:, 128:129], 1.0)
                    if SIM_SAFE_SILU:
                        sg = sb.tile([128, 128], f32, tag="sg")
                        nc.scalar.activation(out=sg[:], in_=z[:], func=AF.Sigmoid)
                        nc.vector.tensor_tensor(out=hs[:, 0:128], in0=z[:], in1=sg[:], op=OP.mult)
                    else:
                        nc.scalar.activation(out=hs[:, 0:128], in_=z[:], func=AF.Silu)

                    nc.tensor.matmul(u_w[:], lhsT=st[:], rhs=hs[:, 0:129],
                                     start=(t == 0), stop=(t == Tw - 1))
                    if DEBUG_DUMP and gt == 0:
                        nc.sync.dma_start(out=dbg["dbg_xg"][:], in_=xg[:])
                        nc.sync.dma_start(out=dbg["dbg_st"][:], in_=st[:])
                        hdbg = sb.tile([128, 128], f32, tag="hdbg")
                        nc.scalar.copy(out=hdbg[:], in_=h_p[:])
                        nc.sync.dma_start(out=dbg["dbg_h"][:], in_=hdbg[:])
                        nc.sync.dma_start(out=dbg["dbg_z"][:], in_=z[:])
                        nc.sync.dma_start(out=dbg["dbg_hs"][:], in_=hs[:])
                    gt += 1

                # ---- window finalize: apply W2, add b2, divide by count ----
                u_s = wp.tile([128, 129], f32, tag="us")
                nc.scalar.copy(out=u_s[:], in_=u_w[:])
                if DEBUG_DUMP and w == 0:
                    nc.sync.dma_start(out=dbg["dbg_us"][:], in_=u_s[:])
                ut_p = ps.tile([128, 128], f32, tag="h")
                nc.tensor.transpose(ut_p[:], u_s[:, 0:128], IDENT[:])
                ut = wp.tile([128, 128], f32, tag="ut")
                nc.scalar.copy(out=ut[:], in_=ut_p[:])
                o_p = ps.tile([128, 128], f32, tag="h")
                nc.tensor.matmul(o_p[:], lhsT=ut[:], rhs=W2[:], start=True, stop=True)

                cm = wp.tile([128, 1], f32, tag="cm")
                nc.vector.tensor_scalar_max(out=cm[:], in0=u_s[:, 128:129], scalar1=1.0)
                inv_c = wp.tile([128, 1], f32, tag="invc")
                nc.vector.reciprocal(out=inv_c[:], in_=cm[:])
                ind = wp.tile([128, 1], f32, tag="ind")
                nc.vector.tensor_tensor(out=ind[:], in0=u_s[:, 128:129], in1=inv_c[:], op=OP.mult)
                ob = wp.tile([128, 128], f32, tag="ob")
                nc.vector.tensor_scalar_mul(out=ob[:], in0=B2T[:], scalar1=ind[:])
                o_s = wp.tile([128, 128], f32, tag="os")
                nc.vector.tensor_scalar_mul(out=o_s[:], in0=o_p[:], scalar1=inv_c[:])
                nc.vector.tensor_tensor(out=o_s[:], in0=o_s[:], in1=ob[:], op=OP.add)
                nc.sync.dma_start(out=out_d[w * WIN:(w + 1) * WIN, :], in_=o_s[:])

    nc.compile()
    return nc


def kernel(x, edge_index, edge_attr, W1, b1, gn_gamma, gn_beta, W2, b2):
    global LAST_EXEC_NS, LAST_RESULTS
    import os
    from concourse.bass_utils import run_bass_kernel_spmd

    x = np.ascontiguousarray(np.asarray(x, dtype=np.float32))
    W1 = np.asarray(W1, dtype=np.float32)
    b1 = np.asarray(b1, dtype=np.float32)
    W2 = np.asarray(W2, dtype=np.float32)
    b2 = np.asarray(b2, dtype=np.float32)
    gn_gamma = np.asarray(gn_gamma, dtype=np.float32)
    gn_beta = np.asarray(gn_beta, dtype=np.float32)

    trivial_affine = bool(
        np.all(gn_gamma == 1.0) and np.all(gn_beta == 0.0)
    )

    T_ws, per_core = _shard(x, np.asarray(edge_index), edge_attr)
    nc = _build_program(T_ws, trivial_affine)

    w1a = np.ascontiguousarray(W1[0:128])        # dst block
    w1b = np.ascontiguousarray(W1[128:256])      # src block
    w1e = np.concatenate([W1[256:260], b1[None, :]], axis=0)  # [5, 128]
    b2t = np.broadcast_to(b2, (128, 128)).copy()
    iota = np.broadcast_to(np.arange(128, dtype=np.float32), (128, 128)).copy()
    ident = np.eye(128, dtype=np.float32)

    shared = {
        "x": x, "w1a": w1a, "w1b": w1b, "w1e": np.ascontiguousarray(w1e),
        "w2": np.ascontiguousarray(W2), "b2t": b2t, "iota": iota, "ident": ident,
    }
    if not trivial_affine:
        shared["gmat"] = np.broadcast_to(gn_gamma, (128, 128)).copy()
        shared["btat"] = np.broadcast_to(gn_beta, (128, 128)).copy()

    in_maps = [dict(shared, **pc) for pc in per_core]
    trace = bool(os.environ.get("BASS_TRACE"))
    res = run_bass_kernel_spmd(nc, in_maps, core_ids=list(range(N_CORES)),
                               trace=trace)
    LAST_EXEC_NS = res.exec_time_ns
    LAST_RESULTS = res

    out = np.empty((N_NODES, OUT_DIM), dtype=np.float32)
    for c in range(N_CORES):
        out[c * NPC:(c + 1) * NPC] = res.results[c]["out"][:NPC]
    return out


# revision 24
# speedup vs baseline: 1.7662x; 1.7662x over previous
"""Trainium2 Bass kernel for MeshConv-style GNN message passing.

Pipeline (per edge e with src s, dst d):
    feat = [x[d], x[s], edge_attr[e]]           # [2*128+4]
    h    = feat @ W1 + b1                       # [128]
    h    = silu(group_norm(h, gamma, beta))     # 8 groups of 16
    msg  = h @ W2 + b2
    out[n] = sum_{e: dst=n} msg[e] / max(count[n], 1)

Sharding: edges sorted by dst, partitioned so each of the 8 cores owns a
contiguous 12,500-node output slice; no cross-core collective.  Nodes are
grouped into 128-node windows, edges padded to 128-edge tiles per window.

Device dataflow (fp16 compute, fp32 accumulation):
 - x[src] rows are gathered per tile with indirect DMA (fp16, 256B rows).
 - x[dst] is NOT gathered: dst lies in the window's 128-row slice x_w, so
   xdT = x_w.T @ S where S[n,e] = (dst[e]==n) is built on-chip (iota
   compare + PE transpose).  S_T doubles as the scatter matrix.
 - MM1 accumulates the dst/src/edge-attr parts into PSUM (b1 folded into
   an augmented edge-attr operand).
 - GroupNorm: per-quad (4 tiles) batched DVE: s1 reduce, center (z1),
   square, s2 reduce; per-window Newton rsqrt (no ACT table thrash);
   per-window single Silu activation instruction.
 - Scatter+MM2 fused by associativity: out_w = (S_T.T @ [h|1]) @ W2,
   accumulated per window in PSUM; counts ride along as a ones column.
"""

import sys

if "/opt/trn_rl_repo" not in sys.path:
    sys.path.insert(0, "/opt/trn_rl_repo")

import numpy as np

N_NODES = 100000
IN_DIM = 128
OUT_DIM = 128
EDGE_DIM = 4
N_GROUPS = 8
GSIZE = IN_DIM // N_GROUPS  # 16
EPS = 1e-5

N_CORES = 8
NPC = N_NODES // N_CORES          # nodes per core (12500)
WIN = 128                         # nodes per window
TE = 128                          # edges per tile

LAST_EXEC_NS = None
LAST_RESULTS = None
# CoreSim lacks Silu; set True to emit Sigmoid+mult instead (sim testing only)
SIM_SAFE_SILU = False


def _shard(x16, edge_index, edge_attr):
    """Sort edges by dst, partition by core / window, pad to tiles."""
    src = np.ascontiguousarray(edge_index[0]).astype(np.int64)
    dst = np.ascontiguousarray(edge_index[1]).astype(np.int64)
    E = src.shape[0]
    ea = np.ascontiguousarray(edge_attr).astype(np.float16)

    order = np.argsort(dst, kind="stable")
    src = src[order]
    dst = dst[order]
    ea = ea[order]

    core = np.minimum(dst // NPC, N_CORES - 1)
    local = dst - core * NPC
    win = local >> 7
    nwin = (NPC + WIN - 1) // WIN  # 98

    cw = core * nwin + win
    counts = np.bincount(cw, minlength=N_CORES * nwin).reshape(N_CORES, nwin)
    T_ws = np.maximum(1, (counts.max(axis=0) + TE - 1) // TE).astype(np.int64)
    total_tiles = int(T_ws.sum())
    cap = total_tiles * TE

    woff = np.zeros(nwin, dtype=np.int64)
    woff[1:] = np.cumsum(T_ws)[:-1] * TE
    cw_starts = np.zeros(N_CORES * nwin, dtype=np.int64)
    cw_starts[1:] = np.cumsum(counts.reshape(-1))[:-1]
    pos_in_cw = np.arange(E, dtype=np.int64) - cw_starts[cw]
    slot = woff[win] + pos_in_cw

    per_core = []
    for c in range(N_CORES):
        m = core == c
        sl = slot[m]
        idx = np.zeros((cap, 1), dtype=np.int32)
        idx[sl, 0] = src[m].astype(np.int32)
        dsh = np.full((cap, 1), -1.0, dtype=np.float16)
        dsh[sl, 0] = (local[m] - (win[m] << 7)).astype(np.float16)
        eat = np.zeros((5, cap), dtype=np.float16)
        eat[4, :] = 1.0
        eat[0:4, sl] = ea[m].T
        per_core.append({"idx": idx, "dsh": dsh, "eat": eat})
    return T_ws, per_core


def _build_program(T_ws, trivial_affine):
    import concourse.bacc as bacc
    import concourse.bass as bass
    from concourse import mybir
    from concourse.tile import TileContext

    f32 = mybir.dt.float32
    f16 = mybir.dt.float16
    i32 = mybir.dt.int32
    AF = mybir.ActivationFunctionType
    OP = mybir.AluOpType
    AX = mybir.AxisListType

    nwin = len(T_ws)
    total_tiles = int(sum(T_ws))
    XPAD = nwin * WIN + (N_CORES - 1) * NPC  # padded x row count (100044+)
    XPAD = ((XPAD + 127) // 128) * 128

    nc = bacc.Bacc()
    x_d = nc.dram_tensor("x16", [XPAD, IN_DIM], f16, kind="ExternalInput")
    base_d = nc.dram_tensor("xw16", [nwin * WIN, IN_DIM], f16, kind="ExternalInput")
    idx_d = nc.dram_tensor("idx", [total_tiles * TE, 1], i32, kind="ExternalInput")
    dsh_d = nc.dram_tensor("dsh", [total_tiles * TE, 1], f16, kind="ExternalInput")
    eat_d = nc.dram_tensor("eat", [5, total_tiles * TE], f16, kind="ExternalInput")
    w1a_d = nc.dram_tensor("w1a", [128, 128], f16, kind="ExternalInput")
    w1b_d = nc.dram_tensor("w1b", [128, 128], f16, kind="ExternalInput")
    w1e_d = nc.dram_tensor("w1e", [5, 128], f16, kind="ExternalInput")
    w2_d = nc.dram_tensor("w2", [128, 128], f16, kind="ExternalInput")
    b2_d = nc.dram_tensor("b2t", [128, 128], f32, kind="ExternalInput")
    iota_d = nc.dram_tensor("iota4", [128, 512], f16, kind="ExternalInput")
    id_d = nc.dram_tensor("ident", [128, 128], f16, kind="ExternalInput")
    id32_d = nc.dram_tensor("ident32", [128, 128], f32, kind="ExternalInput")
    if not trivial_affine:
        gma_d = nc.dram_tensor("gmat", [128, 512], f16, kind="ExternalInput")
        bta_d = nc.dram_tensor("btat", [128, 512], f16, kind="ExternalInput")
    out_d = nc.dram_tensor("out", [nwin * WIN, OUT_DIM], f32, kind="ExternalOutput")

    idx_v = idx_d[:].rearrange("(t p) c -> t p c", p=TE)
    dsh_v = dsh_d[:].rearrange("(t p) c -> t p c", p=TE)

    nq_max = max(int(t + 3) // 4 for t in T_ws)
    with TileContext(nc) as tc:
        with (
            tc.tile_pool(name="const", bufs=1) as cp,
            tc.tile_pool(name="sb", bufs=3) as sb,
            tc.tile_pool(name="keep", bufs=nq_max + 2) as kp,
            tc.tile_pool(name="zz", bufs=2) as zz,
            tc.tile_pool(name="wp", bufs=2) as wp,
            tc.tile_pool(name="p1", bufs=1, space="PSUM") as p1,
            tc.tile_pool(name="p2", bufs=2, space="PSUM") as p2,
            tc.tile_pool(name="pw", bufs=2, space="PSUM") as pw,
        ):
            def cload(dram, shape, tag, dt=f16):
                t = cp.tile(shape, dt, tag=tag)
                nc.sync.dma_start(out=t[:], in_=dram[:])
                return t

            W1A = cload(w1a_d, [128, 128], "c_w1a")
            W1B = cload(w1b_d, [128, 128], "c_w1b")
            W1E = cload(w1e_d, [5, 128], "c_w1e")
            W2 = cload(w2_d, [128, 128], "c_w2")
            B2T = cload(b2_d, [128, 128], "c_b2t", f32)
            IOTA4 = cload(iota_d, [128, 512], "c_iota")
            IDENT = cload(id_d, [128, 128], "c_id")
            IDENT32 = cload(id32_d, [128, 128], "c_id32", f32)
            if not trivial_affine:
                GMAT = cload(gma_d, [128, 512], "c_gma")
                BTAT = cload(bta_d, [128, 512], "c_bta")

            gt = 0
            for w in range(nwin):
                Tw = int(T_ws[w])
                x_w = wp.tile([128, 128], f16, tag="xw")
                nc.sync.dma_start(out=x_w[:], in_=base_d[w * WIN:(w + 1) * WIN, :])
                u_w = pw.tile([128, 129], f32, tag="uw")
                v_all = wp.tile([128, 8 * Tw], f32, tag="vall")
                z_all = zz.tile([128, Tw * 128], f16, tag="zall")
                hs_all = zz.tile([128, Tw * 129], f16, tag="hsall")
                st_tiles = []
                z1_tiles = []

                n_q = (Tw + 3) // 4
                for q in range(n_q):
                    t0 = q * 4
                    QW = min(4, Tw - t0)
                    EQ = QW * 128
                    g0 = gt + t0

                    idx_q = sb.tile([128, QW], i32, tag="idxq")
                    nc.sync.dma_start(
                        out=idx_q[:, :, None],
                        in_=idx_v[g0:g0 + QW].rearrange("k p c -> p k c"),
                    )
                    dsh_q = sb.tile([128, QW], f16, tag="dshq")
                    nc.sync.dma_start(
                        out=dsh_q[:, :, None],
                        in_=dsh_v[g0:g0 + QW].rearrange("k p c -> p k c"),
                    )
                    eat_q = sb.tile([5, EQ], f16, tag="eat")
                    nc.sync.dma_start(
                        out=eat_q[:], in_=eat_d[:, g0 * TE:(g0 + QW) * TE])

                    xs16 = sb.tile([128, EQ], f16, tag="xs16")
                    for k in range(QW):
                        nc.gpsimd.indirect_dma_start(
                            out=xs16[:, k * 128:(k + 1) * 128], out_offset=None,
                            in_=x_d[:],
                            in_offset=bass.IndirectOffsetOnAxis(
                                ap=idx_q[:, k:k + 1], axis=0),
                        )

                    # S_T[e, n] = (dsh[e] == n), one batched compare per quad
                    st_q = kp.tile([128, EQ], f16, tag="st")
                    nc.vector.tensor_tensor(
                        out=st_q[:].rearrange("p (k n) -> p k n", n=128),
                        in0=dsh_q[:, :, None].to_broadcast([128, QW, 128]),
                        in1=IOTA4[:, 0:EQ].rearrange("p (k n) -> p k n", n=128),
                        op=OP.is_equal,
                    )
                    st_tiles.append(st_q)

                    # transposes: xs -> xsT, S_T -> S (shared psum banks)
                    xsT_p = p1.tile([128, EQ], f16, tag="xsTp")
                    s_p = p1.tile([128, EQ], f16, tag="sp")
                    for k in range(QW):
                        sl = slice(k * 128, (k + 1) * 128)
                        nc.tensor.transpose(xsT_p[:, sl], xs16[:, sl], IDENT[:])
                        nc.tensor.transpose(s_p[:, sl], st_q[:, sl], IDENT[:])
                    xsT16 = sb.tile([128, EQ], f16, tag="xsT16")
                    nc.vector.tensor_copy(out=xsT16[:], in_=xsT_p[:])
                    s16 = sb.tile([128, EQ], f16, tag="s16")
                    nc.vector.tensor_copy(out=s16[:], in_=s_p[:])

                    # xdT = x_w.T @ S
                    xdT_p = p2.tile([128, EQ], f32, tag="xdTp")
                    for k in range(QW):
                        sl = slice(k * 128, (k + 1) * 128)
                        nc.tensor.matmul(xdT_p[:, sl], lhsT=x_w[:], rhs=s16[:, sl],
                                         start=True, stop=True)
                    xdT16 = sb.tile([128, EQ], f16, tag="xdT16")
                    nc.scalar.copy(out=xdT16[:], in_=xdT_p[:])

                    # MM1: h = xd@W1a + xs@W1b + ea_aug@W1e_aug
                    h_p = p2.tile([128, EQ], f32, tag="h")
                    for k in range(QW):
                        sl = slice(k * 128, (k + 1) * 128)
                        nc.tensor.matmul(h_p[:, sl], lhsT=xdT16[:, sl], rhs=W1A[:],
                                         start=True, stop=False)
                        nc.tensor.matmul(h_p[:, sl], lhsT=xsT16[:, sl], rhs=W1B[:],
                                         start=False, stop=False)
                        nc.tensor.matmul(h_p[:, sl], lhsT=eat_q[:, sl],
                                         rhs=W1E[:], start=False, stop=True)

                    # GroupNorm stats (batched over the quad)
                    hg = h_p[:].rearrange("p (g c) -> p g c", c=GSIZE)
                    s1 = sb.tile([128, 8 * QW], f32, tag="s1")
                    nc.vector.tensor_reduce(out=s1[:], in_=hg, axis=AX.X, op=OP.add)
                    z1 = kp.tile([128, EQ], f16, tag="z1")
                    nc.vector.scalar_tensor_tensor(
                        out=z1[:].rearrange("p (g c) -> p g c", c=GSIZE),
                        in0=s1[:, :, None].to_broadcast([128, 8 * QW, GSIZE]),
                        scalar=-1.0 / GSIZE, in1=hg, op0=OP.mult, op1=OP.add,
                    )
                    z1_tiles.append(z1)
                    z1sq = sb.tile([128, EQ], f16, tag="z1sq")
                    nc.vector.tensor_tensor(out=z1sq[:], in0=z1[:], in1=z1[:], op=OP.mult)
                    nc.vector.tensor_reduce(
                        out=v_all[:, 8 * t0:8 * t0 + 8 * QW],
                        in_=z1sq[:].rearrange("p (g c) -> p g c", c=GSIZE),
                        axis=AX.X, op=OP.add,
                    )

                # ---- Newton rsqrt over the whole window: inv = rsqrt(v/16+eps)
                SW = 8 * Tw
                v2 = wp.tile([128, SW], f32, tag="v2")
                nc.vector.tensor_scalar(out=v2[:], in0=v_all[:], scalar1=1.0 / GSIZE,
                                        scalar2=EPS, op0=OP.mult, op1=OP.add)
                vh = wp.tile([128, SW], f32, tag="vh")
                nc.vector.tensor_scalar_mul(out=vh[:], in0=v2[:], scalar1=0.5)
                y = wp.tile([128, SW], f32, tag="y")
                # quake initial guess: y0 = bits(0x5f3759df - (bits(v)>>1))
                nc.vector.tensor_scalar(
                    out=y[:].bitcast(i32), in0=v2[:].bitcast(i32), scalar1=1,
                    scalar2=None, op0=OP.logical_shift_right)
                nc.vector.tensor_scalar(
                    out=y[:].bitcast(i32), in0=y[:].bitcast(i32), scalar1=-1,
                    scalar2=0x5F3759DF, op0=OP.mult, op1=OP.add)
                for _ in range(3):
                    a = wp.tile([128, SW], f32, tag="nta")
                    nc.vector.tensor_tensor(out=a[:], in0=y[:], in1=y[:], op=OP.mult)
                    nc.vector.tensor_tensor(out=a[:], in0=a[:], in1=vh[:], op=OP.mult)
                    nc.vector.tensor_scalar(out=a[:], in0=a[:], scalar1=-1.0,
                                            scalar2=1.5, op0=OP.mult, op1=OP.add)
                    nc.vector.tensor_tensor(out=y[:], in0=y[:], in1=a[:], op=OP.mult)
                inv16 = wp.tile([128, SW], f16, tag="inv16")
                nc.vector.tensor_copy(out=inv16[:], in_=y[:])

                # ---- z = z1 * inv ; batched silu; scatter ----
                for q in range(n_q):
                    t0 = q * 4
                    QW = min(4, Tw - t0)
                    nc.vector.tensor_tensor(
                        out=z_all[:, t0 * 128:(t0 + QW) * 128].rearrange(
                            "p (g c) -> p g c", c=GSIZE),
                        in0=z1_tiles[q][:].rearrange("p (g c) -> p g c", c=GSIZE),
                        in1=inv16[:, 8 * t0:8 * (t0 + QW), None].to_broadcast(
                            [128, 8 * QW, GSIZE]),
                        op=OP.mult,
                    )
                if not trivial_affine:
                    for q in range(n_q):
                        t0 = q * 4
                        QW = min(4, Tw - t0)
                        sl = slice(t0 * 128, (t0 + QW) * 128)
                        nc.vector.tensor_tensor(out=z_all[:, sl], in0=z_all[:, sl],
                                                in1=GMAT[:, 0:QW * 128], op=OP.mult)
                        nc.vector.tensor_tensor(out=z_all[:, sl], in0=z_all[:, sl],
                                                in1=BTAT[:, 0:QW * 128], op=OP.add)

                hs_v = hs_all[:].rearrange("p (t c) -> p t c", c=129)
                nc.vector.memset(hs_v[:, :, 128:129], 1.0)
                z_v = z_all[:].rearrange("p (t c) -> p t c", c=128)
                if SIM_SAFE_SILU:
                    sg = zz.tile([128, Tw * 128], f16, tag="sg")
                    nc.scalar.activation(out=sg[:], in_=z_all[:], func=AF.Sigmoid)
                    nc.vector.tensor_tensor(
                        out=hs_v[:, :, 0:128],
                        in0=z_v, in1=sg[:].rearrange("p (t c) -> p t c", c=128),
                        op=OP.mult)
                else:
                    nc.scalar.activation(
                        out=hs_v[:, :, 0:128], in_=z_v, func=AF.Silu)

                for t in range(Tw):
                    nc.tensor.matmul(
                        u_w[:], lhsT=st_tiles[t // 4][:, (t % 4) * 128:(t % 4 + 1) * 128],
                        rhs=hs_v[:, t, 0:129],
                        start=(t == 0), stop=(t == Tw - 1))

                # ---- window finalize: W2, b2, divide by count ----
                u_s = wp.tile([128, 129], f32, tag="us")
                nc.scalar.copy(out=u_s[:], in_=u_w[:])
                ut_p = p2.tile([128, 128], f32, tag="h")
                nc.tensor.transpose(ut_p[:], u_s[:, 0:128], IDENT32[:])
                ut16 = wp.tile([128, 128], f16, tag="ut")
                nc.scalar.copy(out=ut16[:], in_=ut_p[:])
                o_p = p2.tile([128, 128], f32, tag="h")
                nc.tensor.matmul(o_p[:], lhsT=ut16[:], rhs=W2[:], start=True, stop=True)

                cm = wp.tile([128, 1], f32, tag="cm")
                nc.vector.tensor_scalar_max(out=cm[:], in0=u_s[:, 128:129], scalar1=1.0)
                inv_c = wp.tile([128, 1], f32, tag="invc")
                nc.vector.reciprocal(out=inv_c[:], in_=cm[:])
                ind = wp.tile([128, 1], f32, tag="ind")
                nc.vector.tensor_tensor(out=ind[:], in0=u_s[:, 128:129], in1=inv_c[:],
                                        op=OP.mult)
                ob = wp.tile([128, 128], f32, tag="ob")
                nc.vector.tensor_scalar_mul(out=ob[:], in0=B2T[:], scalar1=ind[:])
                o_s = wp.tile([128, 128], f32, tag="os")
                nc.vector.tensor_scalar_mul(out=o_s[:], in0=o_p[:], scalar1=inv_c[:])
                nc.vector.tensor_tensor(out=o_s[:], in0=o_s[:], in1=ob[:], op=OP.add)
                nc.sync.dma_start(out=out_d[w * WIN:(w + 1) * WIN, :], in_=o_s[:])
                gt += Tw

    nc.compile()
    return nc


def _prepare(x, edge_index, edge_attr, W1, b1, gn_gamma, gn_beta, W2, b2):
    x = np.ascontiguousarray(np.asarray(x, dtype=np.float32))
    W1 = np.asarray(W1, dtype=np.float32)
    b1 = np.asarray(b1, dtype=np.float32)
    W2 = np.asarray(W2, dtype=np.float32)
    b2 = np.asarray(b2, dtype=np.float32)
    gn_gamma = np.asarray(gn_gamma, dtype=np.float32)
    gn_beta = np.asarray(gn_beta, dtype=np.float32)

    trivial_affine = bool(np.all(gn_gamma == 1.0) and np.all(gn_beta == 0.0))

    x16 = x.astype(np.float16)
    T_ws, per_core = _shard(x16, np.asarray(edge_index), edge_attr)
    nwin = len(T_ws)
    nc = _build_program(T_ws, trivial_affine)

    XPAD = ((nwin * WIN + (N_CORES - 1) * NPC + 127) // 128) * 128
    x16p = np.zeros((XPAD, IN_DIM), dtype=np.float16)
    x16p[:N_NODES] = x16

    w1a = np.ascontiguousarray(W1[0:128]).astype(np.float16)
    w1b = np.ascontiguousarray(W1[128:256]).astype(np.float16)
    w1e = np.concatenate([W1[256:260], b1[None, :]], axis=0).astype(np.float16)
    b2t = np.broadcast_to(b2, (128, 128)).astype(np.float32).copy()
    iota4 = np.broadcast_to(
        np.tile(np.arange(128, dtype=np.float16), 4), (128, 512)).copy()
    ident = np.eye(128, dtype=np.float16)

    shared = {
        "x16": x16p, "w1a": w1a, "w1b": w1b, "w1e": np.ascontiguousarray(w1e),
        "w2": np.ascontiguousarray(W2).astype(np.float16), "b2t": b2t,
        "iota4": iota4, "ident": ident, "ident32": np.eye(128, dtype=np.float32),
    }
    if not trivial_affine:
        shared["gmat"] = np.broadcast_to(
            np.tile(gn_gamma.astype(np.float16), 4), (128, 512)).copy()
        shared["btat"] = np.broadcast_to(
            np.tile(gn_beta.astype(np.float16), 4), (128, 512)).copy()

    in_maps = []
    for c in range(N_CORES):
        m = dict(shared, **per_core[c])
        m["xw16"] = np.ascontiguousarray(x16p[c * NPC: c * NPC + nwin * WIN])
        in_maps.append(m)
    return nc, in_maps


def kernel(x, edge_index, edge_attr, W1, b1, gn_gamma, gn_beta, W2, b2):
    global LAST_EXEC_NS, LAST_RESULTS
    import os
    from concourse.bass_utils import run_bass_kernel_spmd

    nc, in_maps = _prepare(x, edge_index, edge_attr, W1, b1,
                           gn_gamma, gn_beta, W2, b2)
    trace = bool(os.environ.get("BASS_TRACE"))
    res = run_bass_kernel_spmd(nc, in_maps, core_ids=list(range(N_CORES)),
                               trace=trace)
    LAST_EXEC_NS = res.exec_time_ns
    LAST_RESULTS = res

    out = np.empty((N_NODES, OUT_DIM), dtype=np.float32)
    for c in range(N_CORES):
        out[c * NPC:(c + 1) * NPC] = res.results[c]["out"][:NPC]
    return out
